# revision 12
# baseline (speedup 1.0000x reference)
"""Trainium2 Bass kernel for: out_t = silu(cumsum_t(x)) diff along T.

Reference (T, B, L, D) = (4, 2, 2048, 4096) f32:
    Y = silu(cumsum(x, axis=0)); out = concat([Y[:1], Y[1:] - Y[:-1]])

Strategy: shard L across the 8 NeuronCores (embarrassingly parallel; the
scan is over T=4 only).  Per core a raw-Bass pipeline streams chunks of
128x4096 f16 through SBUF:

  SP  : ALL DMA traffic on the one qSPDynamicHW ring — 1 MiB chunk
        loads (first chunk split per t-slice so the 16 SDMA engines ramp
        sooner) interleaved with 1 MiB chunk stores, each store issued
        the moment its chunk's diffs are done (the store's semaphore
        wait transitively subsumes the following load's slot-free wait,
        so stores never delay loads)
  Pool: copies the x0 slice into col 0 of the [P, 4F] running-sum tile
  DVE : 3 running-sum adds into cols 1..3 (all-f16 so every op runs in
        the 2x_1p high-rate mode), then after the silu 3 in-place
        right-to-left diffs on the output tile (WAR within one engine
        needs no drain; only RAW does)
  ACT : ONE silu per chunk over the whole [P, 4F] tile, written straight
        into the output tile — the scalar engine runs nothing else

Explicit semaphores, one per DMA (no lane arithmetic); cross-engine
deps are standalone sequencer wait_ge instructions.

Both input and output cross HBM as f16 (the host downcasts x and widens
the result back to f32): ~7e-4 l2 rel err, well inside the 2e-2 gate,
cutting HBM traffic from 64 MiB to 32 MiB per core.  The 16 SDMA
engines sustain ~425-475 GB/s/core, so the DMA floor is ~70-79 us.
"""

import sys

if "/opt/trn_rl_repo" not in sys.path:
    sys.path.insert(0, "/opt/trn_rl_repo")

import numpy as np

T, B, L, D = 4, 2, 2048, 4096
NCORES = 8
LS = L // NCORES            # 256 rows of L per core
NPOS = B * LS * D           # 2_097_152 elements per t-slice per core
P = 128                     # SBUF partitions
F = 1024                    # free-dim elements per t-slice per chunk
TF = T * F                  # flat free size of one chunk tile
NCHUNK = NPOS // (P * F)    # 16 chunk iterations per core
NBUF = 11                   # xb slot count (input lookahead)
NOB = 8                     # ob slot count
PPA = 4                     # acc slot count

_NC_CACHE = {}
LAST_RESULT = None
TRACE = False
TRACE_CORES = None
TMPDIR = None


def _build_nc(use_silu: bool = True):
    import concourse.bass as bass
    from concourse import mybir

    f16 = mybir.dt.float16
    act_fn = (
        mybir.ActivationFunctionType.Silu
        if use_silu
        else mybir.ActivationFunctionType.Sigmoid
    )

    nc = bass.Bass("TRN2", debug=False)
    # Chunk-major DRAM layout [NCHUNK, P, T*F] (host repacks): each
    # partition's chunk data is one contiguous 8 KiB run, so every DMA
    # is a straight copy with maximal descriptors.
    x_d = nc.declare_dram_parameter("x", [NCHUNK, P, TF], f16, isOutput=False)
    o_d = nc.declare_dram_parameter("out", [NCHUNK, P, TF], f16, isOutput=True)

    xb = [nc.alloc_sbuf_tensor(f"xb{s}", [P, TF], f16).ap() for s in range(NBUF)]
    ob = [nc.alloc_sbuf_tensor(f"ob{s}", [P, TF], f16).ap() for s in range(NOB)]
    acc = [nc.alloc_sbuf_tensor(f"acc{s}", [P, TF], f16).ap() for s in range(PPA)]

    def col(ap, t):  # t-th F-wide column of a flat [P, 4F] tile
        return ap[:, t * F:(t + 1) * F]

    import contextlib

    with contextlib.ExitStack() as es:
        block = es.enter_context(nc.Block())
        # One semaphore per DMA: thresholds are always ">= 16".
        s_ld = [es.enter_context(nc.semaphore(f"s_ld{i}")) for i in range(NCHUNK)]
        s_st = [es.enter_context(nc.semaphore(f"s_st{i}")) for i in range(NCHUNK)]
        s_l0 = [es.enter_context(nc.semaphore(f"s_l0_{t}")) for t in range(T)]
        s_ls = [es.enter_context(nc.semaphore(f"s_ls{t}")) for t in range(T)]
        s_acc = es.enter_context(nc.semaphore("s_acc"))   # 3 / chunk (adds)
        s_cp = es.enter_context(nc.semaphore("s_cp"))     # 1 / chunk (x0 copy)
        s_act = es.enter_context(nc.semaphore("s_act"))   # 1 / chunk (silu)
        s_out = es.enter_context(nc.semaphore("s_out"))   # 3 / chunk (diffs)
        LAST = NCHUNK - 1

        def wait_slice(eng, i, t):
            # load of chunk i's t-th slice complete
            if i == 0:
                eng.wait_ge(s_l0[t], 16)
            else:
                eng.wait_ge(s_ld[i], 16)

        @block.sync
        def _(sp: bass.BassEngine):
            def emit_load(i):
                if i >= NBUF:
                    j = i - NBUF
                    # xb slot free: DVE adds + Pool x0-copy of chunk j done.
                    sp.wait_ge(s_acc, 3 * (j + 1))
                    sp.wait_ge(s_cp, j + 1)
                if i == 0:
                    # split: smaller first DMAs reach all 16 SDMA engines
                    # (esp. the late-starting ones) sooner
                    for t in range(T):
                        sp.dma_start(
                            out=col(xb[0], t), in_=col(x_d[0], t)
                        ).then_inc(s_l0[t], 16)
                else:
                    sp.dma_start(
                        out=xb[i % NBUF][:], in_=x_d[i]
                    ).then_inc(s_ld[i], 16)

            def emit_store(i):
                # store chunk i the moment DVE's diffs are done.  This
                # wait subsumes any following load's slot-free wait
                # (subs(i) ⟹ silu(i) ⟹ adds/copy(i)), so interleaving
                # stores never delays loads.
                sp.wait_ge(s_out, 3 * (i + 1))
                sp.dma_start(out=o_d[i], in_=ob[i % NOB][:]).then_inc(s_st[i], 16)

            for i in range(NBUF + 1):          # loads 0..11
                emit_load(i)
            for i in range(NBUF + 1, NCHUNK):  # L12,S0,L13,S1,L14,S2,L15,S3
                emit_load(i)
                emit_store(i - NBUF - 1)
            for i in range(NCHUNK - NBUF - 1, LAST):  # S4..S14
                emit_store(i)
            # last chunk: per-slice stores — each output slice leaves as
            # soon as its diff is done, shrinking the end critical path.
            # Slice 0 (= Y0) is untouched by the in-place diffs, so it can
            # go right after the silu; diffs complete in order 3, 2, 1.
            sp.wait_ge(s_act, NCHUNK)
            sp.dma_start(out=col(o_d[LAST], 0), in_=col(ob[LAST % NOB], 0)
                         ).then_inc(s_ls[0], 16)
            for k, t in enumerate((3, 2, 1)):
                sp.wait_ge(s_out, 3 * LAST + k + 1)
                sp.dma_start(out=col(o_d[LAST], t), in_=col(ob[LAST % NOB], t)
                             ).then_inc(s_ls[t], 16)
            for i in range(LAST):
                sp.wait_ge(s_st[i], 16)
            for t in range(T):
                sp.wait_ge(s_ls[t], 16)

        @block.gpsimd
        def _(ge: bass.BassEngine):
            # X0 = x0: copy the x0 slice into col 0 of the acc tile so the
            # scalar engine can silu the whole [P, 4F] tile in ONE shot.
            for i in range(NCHUNK):
                wait_slice(ge, i, 0)
                if i >= PPA:
                    # acc slot free: silu of chunk i-PPA done reading it
                    ge.wait_ge(s_act, i - PPA + 1)
                ge.tensor_copy(col(acc[i % PPA], 0), col(xb[i % NBUF], 0)
                               ).then_inc(s_cp)

        @block.vector
        def _(ve: bass.BassEngine):
            def emit_adds(i):
                xs, a = xb[i % NBUF], acc[i % PPA]
                wait_slice(ve, i, 0)
                if i == 0:
                    ve.wait_ge(s_l0[1], 16)
                if i >= PPA:
                    ve.wait_ge(s_act, i - PPA + 1)  # acc slot free
                ve.tensor_add(col(a, 1), col(xs, 0), col(xs, 1)).then_inc(s_acc)
                # same-engine RAW on the acc chain needs a drain-backed wait
                ve.wait_ge(s_acc, 3 * i + 1)
                if i == 0:
                    ve.wait_ge(s_l0[2], 16)
                ve.tensor_add(col(a, 2), col(a, 1), col(xs, 2)).then_inc(s_acc)
                ve.wait_ge(s_acc, 3 * i + 2)
                if i == 0:
                    ve.wait_ge(s_l0[3], 16)
                ve.tensor_add(col(a, 3), col(a, 2), col(xs, 3)).then_inc(s_acc)

            def emit_subs(i):
                # In-place right-to-left diffs on the output tile: each op
                # only WARs (never RAWs) earlier ops, so no drain waits.
                o = ob[i % NOB]
                ve.wait_ge(s_act, i + 1)  # silu of chunk i drained
                ve.tensor_sub(col(o, 3), col(o, 3), col(o, 2)).then_inc(s_out)
                ve.tensor_sub(col(o, 2), col(o, 2), col(o, 1)).then_inc(s_out)
                ve.tensor_sub(col(o, 1), col(o, 1), col(o, 0)).then_inc(s_out)

            # Software-pipelined: the adds of chunk i+1 run while ACT silus
            # chunk i, so the diffs' s_act wait is satisfied when reached.
            emit_adds(0)
            for i in range(NCHUNK):
                if i + 1 < NCHUNK:
                    emit_adds(i + 1)
                emit_subs(i)

        @block.scalar
        def _(se: bass.BassEngine):
            # ACT is a pure silu stream: one [P, 4F] activation per chunk,
            # written straight into the output tile.
            for i in range(NCHUNK):
                se.wait_ge(s_acc, 3 * i + 3)  # adds of chunk i done
                se.wait_ge(s_cp, i + 1)       # x0 copy of chunk i done
                if i >= NOB:
                    se.wait_ge(s_st[i - NOB], 16)  # ob slot free
                se.activation(ob[i % NOB][:], acc[i % PPA][:], act_fn
                              ).then_inc(s_act)

    return nc


def get_nc(use_silu: bool = True):
    key = ("nc", use_silu)
    if key not in _NC_CACHE:
        _NC_CACHE[key] = _build_nc(use_silu)
    return _NC_CACHE[key]


def kernel(x: np.ndarray) -> np.ndarray:
    global LAST_RESULT
    from concourse.bass_utils import run_bass_kernel_spmd

    nc = get_nc()
    x = np.asarray(x, dtype=np.float32).astype(np.float16)
    # repack each core's shard to the chunk-major [NCHUNK, P, T*F] DRAM
    # layout the kernel uses (contiguous per-partition DMA runs)
    in_maps = [
        {"x": np.ascontiguousarray(
            x[:, :, c * LS : (c + 1) * LS, :]
            .reshape(T, NCHUNK, P, F)
            .transpose(1, 2, 0, 3)
            .reshape(NCHUNK, P, TF)
        )}
        for c in range(NCORES)
    ]
    try:
        res = run_bass_kernel_spmd(
            nc, in_maps, list(range(NCORES)), trace=TRACE, tmpdir=TMPDIR,
            trace_cores=TRACE_CORES,
        )
    except Exception:
        # rare transient NRT_EXEC_UNIT_UNRECOVERABLE; the device recovers
        # on the next execution
        res = run_bass_kernel_spmd(
            nc, in_maps, list(range(NCORES)), trace=TRACE, tmpdir=TMPDIR,
            trace_cores=TRACE_CORES,
        )
    LAST_RESULT = res
    outs = [
        np.asarray(res.results[c]["out"], dtype=np.float32)
        .reshape(NCHUNK, P, T, F)
        .transpose(2, 0, 1, 3)
        .reshape(T, B, LS, D)
        for c in range(NCORES)
    ]
    return np.concatenate(outs, axis=2)


# revision 13
# speedup vs baseline: 1.2193x; 1.2193x over previous
"""Trainium2 Bass kernel for: out_t = silu(cumsum_t(x)) diff along T.

Reference (T, B, L, D) = (4, 2, 2048, 4096) f32:
    Y = silu(cumsum(x, axis=0)); out = concat([Y[:1], Y[1:] - Y[:-1]])

Strategy: shard L across the 8 NeuronCores (embarrassingly parallel; the
scan is over T=4 only).  Per core a raw-Bass pipeline streams chunks of
128x4096 f16 through SBUF:

  SP  : ALL DMA traffic on the one qSPDynamicHW ring.  Each chunk loads
        as TWO DMAs: the x0 slice lands DIRECTLY in col 0 of the
        [P, 4F] running-sum tile (so no engine ever copies it) and
        slices 1..3 land in a [P, 3F] stage tile.  Stores are
        interleaved so each is issued the moment its chunk's diffs are
        done; every store's semaphore wait transitively subsumes the
        following loads' slot-free waits, so stores never delay loads.
  DVE : 3 running-sum adds into cols 1..3 (all-f16 so every op runs in
        the 2x_1p high-rate mode), then after the silu 3 in-place
        right-to-left diffs on the output tile (WAR within one engine
        needs no drain; only RAW does)
  ACT : ONE silu per chunk over the whole [P, 4F] tile, written straight
        into the output tile — the scalar engine runs nothing else

Explicit semaphores, one per DMA (no lane arithmetic); cross-engine
deps are standalone sequencer wait_ge instructions.

Both input and output cross HBM as f16 (the host downcasts x and widens
the result back to f32): ~7e-4 l2 rel err, well inside the 2e-2 gate,
cutting HBM traffic from 64 MiB to 32 MiB per core.  The 16 SDMA
engines sustain ~425-475 GB/s/core (measured per-packet: ~8 ns fixed +
27.1 B/ns), so the DMA floor is ~80 us of fabric time.
"""

import sys

if "/opt/trn_rl_repo" not in sys.path:
    sys.path.insert(0, "/opt/trn_rl_repo")

import numpy as np

T, B, L, D = 4, 2, 2048, 4096
NCORES = 8
LS = L // NCORES            # 256 rows of L per core
NPOS = B * LS * D           # 2_097_152 elements per t-slice per core
P = 128                     # SBUF partitions
F = 1024                    # free-dim elements per t-slice per chunk
TF = T * F                  # flat free size of one chunk tile
NCHUNK = NPOS // (P * F)    # 16 chunk iterations per core
NBUF = 12                   # xb slot count (x1..x3 stage lookahead)
NOB = 8                     # ob slot count
PPA = 6                     # acc slot count (bounds x0-load lookahead)

_NC_CACHE = {}
LAST_RESULT = None
TRACE = False
TRACE_CORES = None
TMPDIR = None


def _build_nc(use_silu: bool = True):
    import concourse.bass as bass
    from concourse import mybir

    f16 = mybir.dt.float16
    act_fn = (
        mybir.ActivationFunctionType.Silu
        if use_silu
        else mybir.ActivationFunctionType.Sigmoid
    )

    nc = bass.Bass("TRN2", debug=False)
    # Chunk-major DRAM layout [NCHUNK, P, T*F] (host repacks): each
    # partition's chunk data is one contiguous 8 KiB run; the two load
    # DMAs split it into a 2 KiB run (x0) and a 6 KiB run (x1..x3).
    x_d = nc.declare_dram_parameter("x", [NCHUNK, P, TF], f16, isOutput=False)
    o_d = nc.declare_dram_parameter("out", [NCHUNK, P, TF], f16, isOutput=True)

    xb = [nc.alloc_sbuf_tensor(f"xb{s}", [P, 3 * F], f16).ap() for s in range(NBUF)]
    ob = [nc.alloc_sbuf_tensor(f"ob{s}", [P, TF], f16).ap() for s in range(NOB)]
    acc = [nc.alloc_sbuf_tensor(f"acc{s}", [P, TF], f16).ap() for s in range(PPA)]

    def col(ap, t):  # t-th F-wide column of a flat tile
        return ap[:, t * F:(t + 1) * F]

    import contextlib

    with contextlib.ExitStack() as es:
        block = es.enter_context(nc.Block())
        # One semaphore per DMA: thresholds are always ">= 16".
        s_la = [es.enter_context(nc.semaphore(f"s_la{i}")) for i in range(NCHUNK)]
        s_lb = [es.enter_context(nc.semaphore(f"s_lb{i}")) for i in range(NCHUNK)]
        s_st = [es.enter_context(nc.semaphore(f"s_st{i}")) for i in range(NCHUNK)]
        s_l0 = [es.enter_context(nc.semaphore(f"s_l0_{t}")) for t in range(1, T)]
        s_ls = [es.enter_context(nc.semaphore(f"s_ls{t}")) for t in range(T)]
        s_acc = es.enter_context(nc.semaphore("s_acc"))   # 3 / chunk (adds)
        s_act = es.enter_context(nc.semaphore("s_act"))   # 1 / chunk (silu)
        s_out = es.enter_context(nc.semaphore("s_out"))   # 3 / chunk (diffs)
        LAST = NCHUNK - 1

        @block.sync
        def _(sp: bass.BassEngine):
            def emit_load_b(i):
                # x0 slice -> acc[i % PPA] col 0.  The tile's previous
                # reader is the silu of chunk i-PPA.
                if i >= PPA:
                    sp.wait_ge(s_act, i - PPA + 1)
                sp.dma_start(
                    out=col(acc[i % PPA], 0), in_=col(x_d[i], 0)
                ).then_inc(s_lb[i], 16)

            def emit_load_a(i):
                # x1..x3 slices -> xb[i % NBUF]; the stage tile's previous
                # reader is the add chain of chunk i-NBUF.
                if i >= NBUF:
                    sp.wait_ge(s_acc, 3 * (i - NBUF + 1))
                if i == 0:
                    # split per slice: smaller first DMAs reach all 16 SDMA
                    # engines (esp. the late-starting ones) sooner
                    for t in range(1, T):
                        sp.dma_start(
                            out=col(xb[0], t - 1), in_=col(x_d[0], t)
                        ).then_inc(s_l0[t - 1], 16)
                else:
                    sp.dma_start(
                        out=xb[i % NBUF][:], in_=x_d[i][:, F:TF]
                    ).then_inc(s_la[i], 16)

            def emit_store(i):
                # store chunk i the moment DVE's diffs are done.  This
                # wait subsumes any following load's slot-free wait
                # (subs(i) ⟹ silu(i) ⟹ adds(i)), so interleaving
                # stores never delays loads.
                sp.wait_ge(s_out, 3 * (i + 1))
                sp.dma_start(out=o_d[i], in_=ob[i % NOB][:]).then_inc(s_st[i], 16)

            # Fill: everything that needs no compute goes out first, in
            # earliest-needed order; then the steady interleave keeps each
            # wait weaker than the next (adds(j) < silu(j) < subs(j) < ...).
            emit_load_b(0)
            emit_load_a(0)
            for i in range(1, PPA):
                emit_load_a(i)
                emit_load_b(i)
            for i in range(PPA, NBUF):
                emit_load_a(i)
            for i in range(NBUF, NCHUNK):    # A12..A15 / B6..B9 / S0..S3
                emit_load_a(i)
                emit_load_b(i - NBUF + PPA)
                emit_store(i - NBUF)
            for i in range(PPA + T, NCHUNK):  # B10..B15 / S4..S9
                emit_load_b(i)
                emit_store(i - PPA)
            for i in range(NCHUNK - PPA, LAST):  # S10..S14
                emit_store(i)
            # last chunk: per-slice stores — each output slice leaves as
            # soon as its diff is done, shrinking the end critical path.
            # Slice 0 (= Y0) is untouched by the in-place diffs, so it can
            # go right after the silu; diffs complete in order 3, 2, 1.
            sp.wait_ge(s_act, NCHUNK)
            sp.dma_start(out=col(o_d[LAST], 0), in_=col(ob[LAST % NOB], 0)
                         ).then_inc(s_ls[0], 16)
            for k, t in enumerate((3, 2, 1)):
                sp.wait_ge(s_out, 3 * LAST + k + 1)
                sp.dma_start(out=col(o_d[LAST], t), in_=col(ob[LAST % NOB], t)
                             ).then_inc(s_ls[t], 16)
            for i in range(LAST):
                sp.wait_ge(s_st[i], 16)
            for t in range(T):
                sp.wait_ge(s_ls[t], 16)

        @block.vector
        def _(ve: bass.BassEngine):
            def emit_adds(i):
                xs, a = xb[i % NBUF], acc[i % PPA]
                # s_lb[i] also transitively covers this acc slot being free
                # (SP waited on silu(i-PPA) before issuing load B).
                ve.wait_ge(s_lb[i], 16)
                if i == 0:
                    ve.wait_ge(s_l0[0], 16)
                else:
                    ve.wait_ge(s_la[i], 16)
                ve.tensor_add(col(a, 1), col(a, 0), col(xs, 0)).then_inc(s_acc)
                # same-engine RAW on the acc chain needs a drain-backed wait
                ve.wait_ge(s_acc, 3 * i + 1)
                if i == 0:
                    ve.wait_ge(s_l0[1], 16)
                ve.tensor_add(col(a, 2), col(a, 1), col(xs, 1)).then_inc(s_acc)
                ve.wait_ge(s_acc, 3 * i + 2)
                if i == 0:
                    ve.wait_ge(s_l0[2], 16)
                ve.tensor_add(col(a, 3), col(a, 2), col(xs, 2)).then_inc(s_acc)

            def emit_subs(i):
                # In-place right-to-left diffs on the output tile: each op
                # only WARs (never RAWs) earlier ops, so no drain waits.
                o = ob[i % NOB]
                ve.wait_ge(s_act, i + 1)  # silu of chunk i drained
                ve.tensor_sub(col(o, 3), col(o, 3), col(o, 2)).then_inc(s_out)
                ve.tensor_sub(col(o, 2), col(o, 2), col(o, 1)).then_inc(s_out)
                ve.tensor_sub(col(o, 1), col(o, 1), col(o, 0)).then_inc(s_out)

            # Software-pipelined: the adds of chunk i+1 run while ACT silus
            # chunk i, so the diffs' s_act wait is satisfied when reached.
            emit_adds(0)
            for i in range(NCHUNK):
                if i + 1 < NCHUNK:
                    emit_adds(i + 1)
                emit_subs(i)

        @block.scalar
        def _(se: bass.BassEngine):
            # ACT is a pure silu stream: one [P, 4F] activation per chunk,
            # written straight into the output tile.
            for i in range(NCHUNK):
                se.wait_ge(s_acc, 3 * i + 3)  # adds of chunk i done
                if i >= NOB:
                    se.wait_ge(s_st[i - NOB], 16)  # ob slot free
                se.activation(ob[i % NOB][:], acc[i % PPA][:], act_fn
                              ).then_inc(s_act)

    return nc


def get_nc(use_silu: bool = True):
    key = ("nc", use_silu)
    if key not in _NC_CACHE:
        _NC_CACHE[key] = _build_nc(use_silu)
    return _NC_CACHE[key]


def kernel(x: np.ndarray) -> np.ndarray:
    global LAST_RESULT
    from concourse.bass_utils import run_bass_kernel_spmd

    nc = get_nc()
    x = np.asarray(x, dtype=np.float32).astype(np.float16)
    # repack each core's shard to the chunk-major [NCHUNK, P, T*F] DRAM
    # layout the kernel uses (contiguous per-partition DMA runs)
    in_maps = [
        {"x": np.ascontiguousarray(
            x[:, :, c * LS : (c + 1) * LS, :]
            .reshape(T, NCHUNK, P, F)
            .transpose(1, 2, 0, 3)
            .reshape(NCHUNK, P, TF)
        )}
        for c in range(NCORES)
    ]
    try:
        res = run_bass_kernel_spmd(
            nc, in_maps, list(range(NCORES)), trace=TRACE, tmpdir=TMPDIR,
            trace_cores=TRACE_CORES,
        )
    except Exception:
        # rare transient NRT_EXEC_UNIT_UNRECOVERABLE; the device recovers
        # on the next execution
        res = run_bass_kernel_spmd(
            nc, in_maps, list(range(NCORES)), trace=TRACE, tmpdir=TMPDIR,
            trace_cores=TRACE_CORES,
        )
    LAST_RESULT = res
    outs = [
        np.asarray(res.results[c]["out"], dtype=np.float32)
        .reshape(NCHUNK, P, T, F)
        .transpose(2, 0, 1, 3)
        .reshape(T, B, LS, D)
        for c in range(NCORES)
    ]
    return np.concatenate(outs, axis=2)


# revision 16
# speedup vs baseline: 1.2483x; 1.0237x over previous
"""Trainium2 Bass kernel for: out_t = silu(cumsum_t(x)) diff along T.

Reference (T, B, L, D) = (4, 2, 2048, 4096) f32:
    Y = silu(cumsum(x, axis=0)); out = concat([Y[:1], Y[1:] - Y[:-1]])

Strategy: shard L across the 8 NeuronCores (embarrassingly parallel; the
scan is over T=4 only).  Per core a raw-Bass pipeline streams chunks of
128x4096 f16 through SBUF:

  SP  : ALL DMA traffic on the one qSPDynamicHW ring.  Each chunk loads
        as TWO DMAs: the x0 slice lands DIRECTLY in col 0 of the
        [P, 4F] running-sum tile (so no engine ever copies it) and
        slices 1..3 land in a [P, 3F] stage tile.  Stores are
        interleaved so each is issued the moment its chunk's diffs are
        done; every store's semaphore wait transitively subsumes the
        following loads' slot-free waits, so stores never delay loads.
  DVE : 3 running-sum adds into cols 1..3 (all-f16 so every op runs in
        the 2x_1p high-rate mode), then after the silu 3 in-place
        right-to-left diffs on the output tile (WAR within one engine
        needs no drain; only RAW does)
  ACT : ONE silu per chunk over the whole [P, 4F] tile, written straight
        into the output tile — the scalar engine runs nothing else

Explicit semaphores, one per DMA (no lane arithmetic); cross-engine
deps are standalone sequencer wait_ge instructions.

Both input and output cross HBM as f16 (the host downcasts x and widens
the result back to f32): ~7e-4 l2 rel err, well inside the 2e-2 gate,
cutting HBM traffic from 64 MiB to 32 MiB per core.  The 16 SDMA
engines sustain ~425-475 GB/s/core (measured per-packet: ~8 ns fixed +
27.1 B/ns), so the DMA floor is ~80 us of fabric time.
"""

import sys

if "/opt/trn_rl_repo" not in sys.path:
    sys.path.insert(0, "/opt/trn_rl_repo")

import numpy as np

T, B, L, D = 4, 2, 2048, 4096
NCORES = 8
LS = L // NCORES            # 256 rows of L per core
NPOS = B * LS * D           # 2_097_152 elements per t-slice per core
P = 128                     # SBUF partitions
F = 1024                    # free-dim elements per t-slice per chunk
TF = T * F                  # flat free size of one chunk tile
NCHUNK = NPOS // (P * F)    # 16 chunk iterations per core
NBUF = 12                   # xb slot count (x1..x3 stage lookahead)
NOB = 8                     # ob slot count
PPA = 6                     # acc slot count (bounds x0-load lookahead)

_NC_CACHE = {}
LAST_RESULT = None
TRACE = False
TRACE_CORES = None
TMPDIR = None


def _build_nc(use_silu: bool = True):
    import concourse.bass as bass
    from concourse import mybir

    f16 = mybir.dt.float16
    act_fn = (
        mybir.ActivationFunctionType.Silu
        if use_silu
        else mybir.ActivationFunctionType.Sigmoid
    )

    nc = bass.Bass("TRN2", debug=False)
    # Chunk-major DRAM layout [NCHUNK, P, T*F] (host repacks): each
    # partition's chunk data is one contiguous 8 KiB run; the two load
    # DMAs split it into a 2 KiB run (x0) and a 6 KiB run (x1..x3).
    x_d = nc.declare_dram_parameter("x", [NCHUNK, P, TF], f16, isOutput=False)
    o_d = nc.declare_dram_parameter("out", [NCHUNK, P, TF], f16, isOutput=True)

    xb = [nc.alloc_sbuf_tensor(f"xb{s}", [P, 3 * F], f16).ap() for s in range(NBUF)]
    ob = [nc.alloc_sbuf_tensor(f"ob{s}", [P, TF], f16).ap() for s in range(NOB)]
    acc = [nc.alloc_sbuf_tensor(f"acc{s}", [P, TF], f16).ap() for s in range(PPA)]

    def col(ap, t):  # t-th F-wide column of a flat tile
        return ap[:, t * F:(t + 1) * F]

    import contextlib

    with contextlib.ExitStack() as es:
        block = es.enter_context(nc.Block())
        # One semaphore per DMA: thresholds are always ">= 16".
        s_la = [es.enter_context(nc.semaphore(f"s_la{i}")) for i in range(NCHUNK)]
        s_lb = [es.enter_context(nc.semaphore(f"s_lb{i}")) for i in range(NCHUNK)]
        s_st = [es.enter_context(nc.semaphore(f"s_st{i}")) for i in range(NCHUNK)]
        s_l0 = [es.enter_context(nc.semaphore(f"s_l0_{t}")) for t in range(1, T)]
        s_ls = [es.enter_context(nc.semaphore(f"s_ls{t}")) for t in range(T)]
        s_acc = es.enter_context(nc.semaphore("s_acc"))   # 3 / chunk (adds)
        s_act = es.enter_context(nc.semaphore("s_act"))   # 1 / chunk (silu)
        s_out = es.enter_context(nc.semaphore("s_out"))   # 3 / chunk (diffs)
        LAST = NCHUNK - 1

        @block.sync
        def _(sp: bass.BassEngine):
            def emit_load_b(i):
                # x0 slice -> acc[i % PPA] col 0.  The tile's previous
                # reader is the silu of chunk i-PPA.
                if i >= PPA:
                    sp.wait_ge(s_act, i - PPA + 1)
                sp.dma_start(
                    out=col(acc[i % PPA], 0), in_=col(x_d[i], 0)
                ).then_inc(s_lb[i], 16)

            def emit_load_a(i):
                # x1..x3 slices -> xb[i % NBUF]; the stage tile's previous
                # reader is the add chain of chunk i-NBUF.
                if i >= NBUF:
                    sp.wait_ge(s_acc, 3 * (i - NBUF + 1))
                if i == 0:
                    # split per slice: smaller first DMAs reach all 16 SDMA
                    # engines (esp. the late-starting ones) sooner
                    for t in range(1, T):
                        sp.dma_start(
                            out=col(xb[0], t - 1), in_=col(x_d[0], t)
                        ).then_inc(s_l0[t - 1], 16)
                else:
                    sp.dma_start(
                        out=xb[i % NBUF][:], in_=x_d[i][:, F:TF]
                    ).then_inc(s_la[i], 16)

            # Fill: everything that needs no compute goes out first, in
            # earliest-needed order; then the steady interleave keeps each
            # wait weaker than the next (adds(j) < silu(j)).  Stores live
            # on ACT's separate HWDGE ring, so the deep load sprint here
            # never head-of-line-blocks a store.
            emit_load_b(0)
            emit_load_a(0)
            for i in range(1, PPA):
                emit_load_a(i)
                emit_load_b(i)
            for i in range(PPA, NBUF):
                emit_load_a(i)
            for i in range(NBUF, NCHUNK):    # A12..A15 / B6..B9
                emit_load_a(i)
                emit_load_b(i - NBUF + PPA)
            for i in range(PPA + T, NCHUNK):  # B10..B15
                emit_load_b(i)

        @block.vector
        def _(ve: bass.BassEngine):
            def emit_adds(i):
                xs, a = xb[i % NBUF], acc[i % PPA]
                # s_lb[i] also transitively covers this acc slot being free
                # (SP waited on silu(i-PPA) before issuing load B).
                ve.wait_ge(s_lb[i], 16)
                if i == 0:
                    ve.wait_ge(s_l0[0], 16)
                else:
                    ve.wait_ge(s_la[i], 16)
                ve.tensor_add(col(a, 1), col(a, 0), col(xs, 0)).then_inc(s_acc)
                # same-engine RAW on the acc chain needs a drain-backed wait
                ve.wait_ge(s_acc, 3 * i + 1)
                if i == 0:
                    ve.wait_ge(s_l0[1], 16)
                ve.tensor_add(col(a, 2), col(a, 1), col(xs, 1)).then_inc(s_acc)
                ve.wait_ge(s_acc, 3 * i + 2)
                if i == 0:
                    ve.wait_ge(s_l0[2], 16)
                ve.tensor_add(col(a, 3), col(a, 2), col(xs, 2)).then_inc(s_acc)

            def emit_subs(i):
                # In-place right-to-left diffs on the output tile: each op
                # only WARs (never RAWs) earlier ops, so no drain waits.
                o = ob[i % NOB]
                ve.wait_ge(s_act, i + 1)  # silu of chunk i drained
                ve.tensor_sub(col(o, 3), col(o, 3), col(o, 2)).then_inc(s_out)
                ve.tensor_sub(col(o, 2), col(o, 2), col(o, 1)).then_inc(s_out)
                ve.tensor_sub(col(o, 1), col(o, 1), col(o, 0)).then_inc(s_out)

            # Software-pipelined: the adds of chunk i+1 run while ACT silus
            # chunk i, so the diffs' s_act wait is satisfied when reached.
            emit_adds(0)
            for i in range(NCHUNK):
                if i + 1 < NCHUNK:
                    emit_adds(i + 1)
                emit_subs(i)

        @block.scalar
        def _(se: bass.BassEngine):
            # ACT: one [P, 4F] silu per chunk, written straight into the
            # output tile, plus the store issues on its own HWDGE ring
            # (qActDynamicHW).  The store of chunk i is issued one
            # iteration LATE (after the silu of chunk i+1): by then DVE's
            # diffs of chunk i are long done, so ACT never stalls waiting
            # on the vector engine.
            def emit_store(i):
                se.wait_ge(s_out, 3 * (i + 1))  # diffs of chunk i done
                se.dma_start(out=o_d[i], in_=ob[i % NOB][:]).then_inc(s_st[i], 16)

            for i in range(NCHUNK):
                se.wait_ge(s_acc, 3 * i + 3)  # adds of chunk i done
                if i >= NOB:
                    se.wait_ge(s_st[i - NOB], 16)  # ob slot free
                se.activation(ob[i % NOB][:], acc[i % PPA][:], act_fn
                              ).then_inc(s_act)
                if i >= 1:
                    emit_store(i - 1)
            # last chunk: per-slice stores — each output slice leaves as
            # soon as its diff is done, shrinking the end critical path.
            # Slice 0 (= Y0) is untouched by the in-place diffs, so it can
            # go right after the silu; diffs complete in order 3, 2, 1.
            se.wait_ge(s_act, NCHUNK)  # own silu(LAST) write drained
            se.dma_start(out=col(o_d[LAST], 0), in_=col(ob[LAST % NOB], 0)
                         ).then_inc(s_ls[0], 16)
            for k, t in enumerate((3, 2, 1)):
                se.wait_ge(s_out, 3 * LAST + k + 1)
                se.dma_start(out=col(o_d[LAST], t), in_=col(ob[LAST % NOB], t)
                             ).then_inc(s_ls[t], 16)
            for i in range(LAST):
                se.wait_ge(s_st[i], 16)
            for t in range(T):
                se.wait_ge(s_ls[t], 16)

    return nc


def get_nc(use_silu: bool = True):
    key = ("nc", use_silu)
    if key not in _NC_CACHE:
        _NC_CACHE[key] = _build_nc(use_silu)
    return _NC_CACHE[key]


def kernel(x: np.ndarray) -> np.ndarray:
    global LAST_RESULT
    from concourse.bass_utils import run_bass_kernel_spmd

    nc = get_nc()
    x = np.asarray(x, dtype=np.float32).astype(np.float16)
    # repack each core's shard to the chunk-major [NCHUNK, P, T*F] DRAM
    # layout the kernel uses (contiguous per-partition DMA runs)
    in_maps = [
        {"x": np.ascontiguousarray(
            x[:, :, c * LS : (c + 1) * LS, :]
            .reshape(T, NCHUNK, P, F)
            .transpose(1, 2, 0, 3)
            .reshape(NCHUNK, P, TF)
        )}
        for c in range(NCORES)
    ]
    try:
        res = run_bass_kernel_spmd(
            nc, in_maps, list(range(NCORES)), trace=TRACE, tmpdir=TMPDIR,
            trace_cores=TRACE_CORES,
        )
    except Exception:
        # rare transient NRT_EXEC_UNIT_UNRECOVERABLE; the device recovers
        # on the next execution
        res = run_bass_kernel_spmd(
            nc, in_maps, list(range(NCORES)), trace=TRACE, tmpdir=TMPDIR,
            trace_cores=TRACE_CORES,
        )
    LAST_RESULT = res
    outs = [
        np.asarray(res.results[c]["out"], dtype=np.float32)
        .reshape(NCHUNK, P, T, F)
        .transpose(2, 0, 1, 3)
        .reshape(T, B, LS, D)
        for c in range(NCORES)
    ]
    return np.concatenate(outs, axis=2)
